# revision 75
# baseline (speedup 1.0000x reference)
"""Trainium2 Bass kernel for nn_GCMC (GNN message passing / GCMC scoring).

Strategy: row-shard users AND items across 8 NeuronCores (256 padded rows
each), replicate the small weights. On-chip compute is fp16 (8x less
quantization error than bf16; full PE rate for the cat-layer matmul),
except the message-passing hot loop which runs fp8e4m3 DoubleRow
(pre-activations scaled x16, M matrices x1024; the psum scale is removed
by the relu activation). Side-branch BatchNorm scale/shift are a pure
function of the inputs and are precomputed on the host.

Exactly TWO collectives on the CC stream:
  AG1: both sides' projected features preT, fp8           (164KB in)
  AG3: pre-BN y_v + both sides' cat-BN partial sums as
       fp16 hi/lo pairs (f32-precision sums; fp16-rounded sums make
       E[x^2]-mu^2 cancel catastrophically), fp16          (39.6KB in)
The u-side cat stats ride AG3 so embed_u/t1 compute during the y_v
read-back; embed_v activates per 512-col chunk matching score v-tiles.

All host-side prep (pad/cast/transpose) repacks tensors partition-major
so every device DMA moves multi-KB contiguous lines per partition.
Projections for both sides share one 512-wide moving tile and pair two
relations per 128-wide stationary, quartering instruction count. Gather
read-backs and final-phase loads ride the otherwise idle GPSIMD DMA
queue so the SP/ACT FIFOs and the scalar engine never block the
critical path. Score outputs store as fp16 and are cast to f32 on host.
"""
import sys
if '/opt/trn_rl_repo' not in sys.path:
    sys.path.insert(0, '/opt/trn_rl_repo')

import numpy as np

import concourse.bass as bass
import concourse.bacc as bacc
import concourse.mybir as mybir
import concourse.tile as tile
from concourse import bass_utils

F16 = mybir.dt.float16
F32 = mybir.dt.float32
F8 = mybir.dt.float8e4
AF = mybir.ActivationFunctionType
ALU = mybir.AluOpType
AXX = mybir.AxisListType.X
DR = mybir.MatmulPerfMode.DoubleRow
PRE_SCALE = 16.0     # fp8 scale for staged pre activations
M_SCALE = 1024.0     # fp8 scale for the M matrices

U = V = F = 2000
R, H, O, SH, SF = 5, 64, 75, 64, 128
UP = 2048            # padded U/V/F
S = 256              # rows per core
NC = 8
KT = 16              # 128-row k-tiles over the padded 2048 contraction dims
EPS = 1e-5
CAT_BLKS = 6         # 768 = 6*128 rows of (padded) cat dim; valid rows: 704
NTILES = [(0, 512), (512, 512), (1024, 512), (1536, 464)]  # score v-tiles
SCOLS = R * H        # 320 stage cols: preT only (side BN stats host-computed)
Y3 = S + 8           # 264: y_v (256) + v & u cat-BN sums as fp16 hi/lo pairs

_CACHE = {}


def _build():
    nc = bacc.Bacc("TRN2", target_bir_lowering=False, debug=False,
                   num_devices=NC)

    def din(name, shape, dt):
        return nc.dram_tensor(name, list(shape), dt, kind="ExternalInput").ap()

    fT_d = din("fT", (128, KT, 2 * S), F16)      # [p, k, v256|u256]
    w2_d = din("w2", (128, KT, R * H), F16)      # [p, k, r*64+h]
    muT_d = din("muT", (R, 128, KT, S), F8)      # pre-scaled by M_SCALE
    mvT_d = din("mvT", (R, 128, KT, S), F8)
    q_d = din("q", (O, R, O), F16)
    sfuT_d = din("sfuT", (SF, S), F16)
    sfvT_d = din("sfvT", (SF, S), F16)
    wside_d = din("wside", (SF, 2, SH), F16)
    wcat_d = din("wcat", (128, 2, CAT_BLKS, O), F16)
    gbs_d = din("gb_side", (SH, 4), F32)
    gbc_d = din("gb_cat", (O, 4), F32)
    ident_d = din("ident", (128, 128), F16)
    mask_d = din("mask", (SH, S), F16)

    score_d = nc.dram_tensor("score", [R, S, V], F16, kind="ExternalOutput").ap()

    with tile.TileContext(nc) as tc:
        with tc.tile_pool(name="const", bufs=1) as const_p, \
             tc.tile_pool(name="big", bufs=1) as big_p, \
             tc.tile_pool(name="mstream", bufs=5) as m_p, \
             tc.tile_pool(name="agload", bufs=1) as ag_p, \
             tc.tile_pool(name="small", bufs=1) as sm_p, \
             tc.tile_pool(name="scoresb", bufs=5) as sc_p, \
             tc.tile_pool(name="psmm", bufs=4, space="PSUM") as psmm, \
             tc.tile_pool(name="pssc", bufs=4, space="PSUM") as pssc, \
             tc.tile_pool(name="dram", bufs=1, space="DRAM") as dram_p:

            replica = [list(range(NC))]

            # ============ input loads (SP + ACT queues) ============
            sfvT_sb = const_p.tile([SF, S], F16)
            nc.sync.dma_start(sfvT_sb[:], sfvT_d)
            sfuT_sb = const_p.tile([SF, S], F16)
            nc.sync.dma_start(sfuT_sb[:], sfuT_d)
            wside_sb = const_p.tile([SF, 2, SH], F16)
            nc.sync.dma_start(wside_sb[:], wside_d)
            # w2/fT split across both queues so they get full DMA bandwidth
            # before the bulk M loads start on ACT.
            w2_sb = big_p.tile([128, KT, R * H], F16)
            nc.sync.dma_start(w2_sb[:, 0:KT // 2], w2_d[:, 0:KT // 2])
            nc.scalar.dma_start(w2_sb[:, KT // 2:], w2_d[:, KT // 2:])
            fT_sb = big_p.tile([128, KT, 2 * S], F16)
            nc.sync.dma_start(fT_sb[:, 0:KT // 2], fT_d[:, 0:KT // 2])
            nc.scalar.dma_start(fT_sb[:, KT // 2:], fT_d[:, KT // 2:])
            ident = const_p.tile([128, 128], F16)
            nc.sync.dma_start(ident[:], ident_d)
            mask_sb = const_p.tile([SH, S], F16)
            nc.sync.dma_start(mask_sb[:], mask_d)
            gbs_sb = const_p.tile([SH, 4], F32)
            nc.sync.dma_start(gbs_sb[:], gbs_d)
            gbc_sb = const_p.tile([O, 4], F32)
            nc.sync.dma_start(gbc_sb[:], gbc_d)
            wcat_sb = const_p.tile([128, 2, CAT_BLKS, O], F16)
            nc.sync.dma_start(wcat_sb[:], wcat_d)
            q_sb = const_p.tile([O, R, O], F16)
            nc.sync.dma_start(q_sb[:], q_d)
            eps_t = const_p.tile([128, 1], F32)
            nc.vector.memset(eps_t[:], EPS)

            # ============ bulk M loads (ACT queue, start immediately) ====
            muT_sb = [m_p.tile([128, KT, S], F8, tag="muT", name=f"muT_{r}")
                      for r in range(R)]
            mvT_sb = [m_p.tile([128, KT, S], F8, tag="mvT", name=f"mvT_{r}")
                      for r in range(R)]
            for r in range(R):
                nc.scalar.dma_start(muT_sb[r][:], muT_d[r])
            for r in range(R):
                nc.scalar.dma_start(mvT_sb[r][:], mvT_d[r])

            # ============ collective buffers ============
            # both sides' pre go out in ONE AllGather (payloads are ready
            # together; merging drops one ~8us fixed collective cost)
            ag_in = dram_p.tile([2, 2, 128, SCOLS], F8, name="ag_in")
            ag_out = dram_p.tile([NC, 2, 2, 128, SCOLS], F8,
                                 addr_space="Shared", name="ag_out")
            ag3_in = dram_p.tile([O, Y3], F16, name="ag3_in")
            ag3_out = dram_p.tile([NC, O, Y3], F16, addr_space="Shared",
                                  name="ag3_out")
            # u-side cat-BN sums go out early in a tiny gather that fires
            # under hidden_v, so u stats/embed_u/t1 are off the tail path
            ag3a_in = dram_p.tile([O, 8], F16, name="ag3a_in")
            ag3a_out = dram_p.tile([NC, O, 8], F16, addr_space="Shared",
                                   name="ag3a_out")

            # catT: 6 blocks of [128, S] fp16 per side (u=0, v=1)
            catT = [[big_p.tile([128, S], F16, name=f"catT_{sd}_{b}")
                     for b in range(CAT_BLKS)] for sd in range(2)]
            stage = [big_p.tile([128, 2, SCOLS], F8, name=f"stage_{sd}")
                     for sd in range(2)]
            rh_scale = const_p.tile([H, 1], F32)
            nc.vector.memset(rh_scale[:], 1.0 / (PRE_SCALE * M_SCALE))

            def cat_slot(base, r):
                row = base + r * H
                return row // 128, row % 128

            # ============ side branches ============
            # BN stats for the side branch depend only on inputs+weights, so
            # the host precomputes scale/shift (gb_side) - no gather needed.
            s_loc = sm_p.tile([SH, 2, S], F32)

            def side_branch(sd, sfT):
                ps_s = psmm.tile([SH, S], F32, tag="mm", name="ps_side")
                nc.tensor.matmul(ps_s[:], wside_sb[:, sd, :], sfT[:],
                                 start=True, stop=True)
                nc.vector.tensor_copy(s_loc[:, sd, :], ps_s[:])
                nc.scalar.activation(catT[sd][5][0:SH, :], s_loc[:, sd, :],
                                     AF.Relu,
                                     bias=gbs_sb[:, 2 * sd + 1:2 * sd + 2],
                                     scale=gbs_sb[:, 2 * sd:2 * sd + 1])
                nc.vector.tensor_mul(catT[sd][5][0:SH, :],
                                     catT[sd][5][0:SH, :], mask_sb[:])

            side_branch(1, sfvT_sb)
            side_branch(0, sfuT_sb)

            # ============ projections: both sides, paired relations ======
            # psum[rp] [128|64, 512] = [W[2rp]|W[2rp+1]]^T @ [fvT|fuT]
            RPAIRS = [(0, 2), (2, 2), (4, 1)]  # (first r, count)
            ps_rp = []
            for rp, (r0, cnt) in enumerate(RPAIRS):
                ps = psmm.tile([cnt * H, 2 * S], F32, tag="mm",
                               name=f"ps_proj{rp}")
                for k in range(KT):
                    nc.tensor.matmul(ps[:],
                                     w2_sb[:, k, r0 * H:(r0 + cnt) * H],
                                     fT_sb[:, k, :],
                                     start=(k == 0), stop=(k == KT - 1))
                ps_rp.append(ps)
            # copy psum -> catT proj rows for both sides (frees psums)
            for sd in range(2):  # v cols live in 0:S, u cols in S:2S
                col = S if sd == 0 else 0
                for rp, (r0, cnt) in enumerate(RPAIRS):
                    for j in range(cnt):
                        blk, off = cat_slot(320, r0 + j)
                        nc.vector.tensor_copy(
                            catT[sd][blk][off:off + H, :],
                            ps_rp[rp][j * H:(j + 1) * H, col:col + S])

            # transpose preT -> natural [v, h] chunks, stage, gather
            def stage_side(sd):
                for r in range(R):
                    blk, off = cat_slot(320, r)
                    for ch in range(2):
                        ps_tp = psmm.tile([128, H], F16, tag="mm", name="ps_tp")
                        nc.tensor.transpose(
                            ps_tp[:],
                            catT[sd][blk][off:off + H, ch * 128:(ch + 1) * 128],
                            ident[off:off + H, off:off + H])
                        nc.vector.tensor_scalar_mul(
                            stage[sd][:, ch, r * H:(r + 1) * H], ps_tp[:],
                            PRE_SCALE)
                nc.sync.dma_start(ag_in[sd].rearrange("c p j -> p c j"),
                                  stage[sd][:])

            stage_side(1)
            stage_side(0)
            nc.gpsimd.collective_compute("AllGather", ALU.bypass,
                                         replica_groups=replica,
                                         ins=[ag_in.opt()],
                                         outs=[ag_out.opt()])

            # ============ gathered pre-activations ============
            # agall[sd] [128, NC, 2, SCOLS]; k-chunk kk -> [:, kk//2, kk%2, :]
            agall = [ag_p.tile([128, NC, 2, SCOLS], F8, name=f"agall{sd}")
                     for sd in range(2)]

            def load_agall(sd):
                # v-gather reads on SP queue; u-gather reads on the otherwise
                # idle GPSIMD queue so neither the SP FIFO (ag3 stages) nor
                # the scalar engine (hidden relu activations) is blocked.
                eng = nc.sync if sd == 1 else nc.gpsimd
                for c in range(NC):
                    eng.dma_start(
                        agall[sd][:, c],
                        ag_out[c, sd].rearrange("ch p j -> p ch j"))

            # ============ BN helpers ============
            def bn_from_sums(sums, sumsq, g_col, b_col, n, P, W=1):
                mu = sm_p.tile([P, W], F32, tag="bn_mu", name="bn_mu")
                nc.vector.tensor_scalar_mul(mu[:], sums[:], 1.0 / n)
                e2 = sm_p.tile([P, W], F32, tag="bn_e2", name="bn_e2")
                nc.vector.tensor_scalar_mul(e2[:], sumsq[:], 1.0 / n)
                var = sm_p.tile([P, W], F32, tag="bn_var", name="bn_var")
                nc.vector.tensor_mul(var[:], mu[:], mu[:])
                nc.vector.tensor_sub(var[:], e2[:], var[:])
                std = sm_p.tile([P, W], F32, tag="bn_std", name="bn_std")
                nc.scalar.activation(std[:], var[:], AF.Sqrt, bias=eps_t[0:P, :])
                rstd = sm_p.tile([P, W], F32, tag="bn_rstd", name="bn_rstd")
                nc.vector.reciprocal(rstd[:], std[:])
                scale = sm_p.tile([P, W], F32, tag="bn_scale", name="bn_scale")
                nc.vector.tensor_mul(scale[:], g_col, rstd[:])
                shift = sm_p.tile([P, W], F32, tag="bn_shift", name="bn_shift")
                nc.vector.tensor_mul(shift[:], mu[:], scale[:])
                nc.vector.tensor_sub(shift[:], b_col, shift[:])
                return scale, shift

            # ============ hidden: relu(pre_all^T @ MT) -> catT rows 0:320 ====
            # fp8 DoubleRow: each matmul consumes a (c, ch) k-tile PAIR at
            # double rate; psum carries PRE_SCALE*M_SCALE, removed by the
            # relu activation's scale.
            def hidden_side(sd, osd, mT):
                for r in range(R):
                    ps_h = psmm.tile([H, S], F32, tag="mm", name="ps_h")
                    for c in range(NC):
                        nc.tensor.matmul(
                            ps_h[:],
                            agall[osd][:, c, :, r * H:(r + 1) * H],
                            mT[r][:, 2 * c:2 * c + 2, :],
                            start=(c == 0), stop=(c == NC - 1),
                            perf_mode=DR)
                    blk, off = cat_slot(0, r)
                    nc.scalar.activation(catT[sd][blk][off:off + H, :],
                                         ps_h[:], AF.Relu, scale=rh_scale[:])

            # ============ cat matmul (fp16) + y stats ============
            ysb = sm_p.tile([O, 2, S], F32)
            junk_y = sm_p.tile([O, 2, S], F32, name="junk_y")
            ag3_sb = sm_p.tile([O, Y3], F16)
            ag3a_sb = sm_p.tile([O, 8], F16)
            nc.vector.memset(ag3a_sb[:, 4:8], 0.0)
            nc.vector.memset(ag3_sb[:, S + 4:Y3], 0.0)

            def cat_side(sd):
                # f32 sums ride the fp16 gather as hi/lo pairs: the BN
                # variance E[x^2]-mu^2 cancels catastrophically with
                # fp16-rounded sums (relu'd features: mean >> std).
                # cols: 256:258 v-sums hi, 258:260 v lo, 260:262 u hi,
                # 262:264 u lo.
                dst = ag3a_sb if sd == 0 else ag3_sb
                base = 0 if sd == 0 else S
                ps_y = psmm.tile([O, S], F32, tag="mm", name="ps_y")
                for b in range(CAT_BLKS):
                    kk = 128 if b < 5 else 64
                    nc.tensor.matmul(ps_y[:], wcat_sb[0:kk, sd, b, :],
                                     catT[sd][b][0:kk, :],
                                     start=(b == 0), stop=(b == CAT_BLKS - 1))
                if sd == 0:
                    nc.vector.tensor_copy(ysb[:, 0, :], ps_y[:])
                else:
                    # y_v goes straight psum -> fp16 payload on the scalar
                    # engine; sums read the psum directly on vector.
                    nc.scalar.copy(ag3_sb[:, 0:S], ps_y[:])
                nc.scalar.activation(junk_y[:, sd, :], ps_y[:], AF.Square)
                yss = sm_p.tile([O, 2], F32, tag=f"yss_{sd}", name="yss")
                nc.vector.reduce_sum(yss[:, 0:1], ps_y[:], axis=AXX)
                nc.vector.reduce_sum(yss[:, 1:2], junk_y[:, sd, :], axis=AXX)
                hi_f = sm_p.tile([O, 2], F32, tag=f"hi_f{sd}", name="hi_f")
                lo_f = sm_p.tile([O, 2], F32, tag=f"lo_f{sd}", name="lo_f")
                nc.vector.tensor_copy(dst[:, base:base + 2], yss[:])
                nc.vector.tensor_copy(hi_f[:], dst[:, base:base + 2])
                nc.vector.tensor_sub(lo_f[:], yss[:], hi_f[:])
                nc.vector.tensor_copy(dst[:, base + 2:base + 4], lo_f[:])

            def half_stats(yfx, c0, c1, g_col, b_col):
                s1 = sm_p.tile([O, 1], F32, tag=f"s1_{c0}", name="s1")
                q1 = sm_p.tile([O, 1], F32, tag=f"q1_{c0}", name="q1")
                sl = sm_p.tile([O, 1], F32, tag=f"sl_{c0}", name="sl")
                ql = sm_p.tile([O, 1], F32, tag=f"ql_{c0}", name="ql")
                nc.vector.reduce_sum(s1[:], yfx[:, :, c0], axis=AXX)
                nc.vector.reduce_sum(q1[:], yfx[:, :, c0 + 1], axis=AXX)
                nc.vector.reduce_sum(sl[:], yfx[:, :, c1], axis=AXX)
                nc.vector.reduce_sum(ql[:], yfx[:, :, c1 + 1], axis=AXX)
                nc.vector.tensor_add(s1[:], s1[:], sl[:])
                nc.vector.tensor_add(q1[:], q1[:], ql[:])
                return bn_from_sums(s1, q1, g_col, b_col, U, O)

            # ============ main sequence ============
            load_agall(1)
            load_agall(0)
            hidden_side(0, 1, muT_sb)
            cat_side(0)
            nc.sync.dma_start(ag3a_in[:], ag3a_sb[:])
            nc.gpsimd.collective_compute("AllGather", ALU.bypass,
                                         replica_groups=replica,
                                         ins=[ag3a_in.opt()],
                                         outs=[ag3a_out.opt()])
            hidden_side(1, 0, mvT_sb)
            # u stats/embed_u complete under hidden_v (vector/scalar only)
            yfa = sm_p.tile([O, NC, 8], F16, name="yfa")
            nc.gpsimd.dma_start(yfa[:], ag3a_out.rearrange("c p j -> p c j"))
            sc_u, sh_u = half_stats(yfa, 0, 2, gbc_sb[:, 0:1], gbc_sb[:, 2:3])
            embed_u = sm_p.tile([O, S], F16)
            nc.scalar.activation(embed_u[:], ysb[:, 0, :],
                                 AF.Relu, bias=sh_u[:], scale=sc_u[:])
            cat_side(1)
            nc.sync.dma_start(ag3_in[:], ag3_sb[:])
            nc.gpsimd.collective_compute("AllGather", ALU.bypass,
                                         replica_groups=replica,
                                         ins=[ag3_in.opt()],
                                         outs=[ag3_out.opt()])
            # t1 matmuls fill the AG3 transfer window
            t1s = []
            for r in range(R):
                ps_t1 = psmm.tile([O, S], F32, tag="mm", name="ps_t1")
                nc.tensor.matmul(ps_t1[:], q_sb[:, r, :], embed_u[:],
                                 start=True, stop=True)
                t1 = sm_p.tile([O, S], F16, tag=f"t1_{r}", name=f"t1_{r}")
                nc.vector.tensor_copy(t1[:], ps_t1[:])
                t1s.append(t1)

            # ============ v stats -> embed_v ============
            # sums columns land first; embed_v activates per 512-col chunk,
            # matching the score v-tiles.
            yf = sm_p.tile([O, NC, Y3], F16, name="yf")
            nc.gpsimd.dma_start(yf[:, :, S:],
                                ag3_out[:, :, S:].rearrange("c p j -> p c j"))
            embed_v = sm_p.tile([O, UP], F16)
            sc_v, sh_v = half_stats(yf, S, S + 2, gbc_sb[:, 1:2],
                                    gbc_sb[:, 3:4])
            embed_v4 = embed_v.rearrange("p (c u) -> p c u", c=NC)
            for j in range(4):
                nc.gpsimd.dma_start(
                    yf[:, 2 * j:2 * j + 2, 0:S],
                    ag3_out[2 * j:2 * j + 2, :, 0:S]
                    .rearrange("c p j -> p c j"))
                nc.scalar.activation(
                    embed_v4[:, 2 * j:2 * j + 2, :],
                    yf[:, 2 * j:2 * j + 2, 0:S],
                    AF.Relu, bias=sh_v[:], scale=sc_v[:])

            # ============ score ============
            for r in range(R):
                for ch in range(2):
                    out_sb = sc_p.tile([128, V], F16, tag="osb", name="out_sb")
                    for i, (n0, nn) in enumerate(NTILES):
                        pool = pssc if i % 2 == 0 else psmm
                        ps_sc = pool.tile([128, 512], F32,
                                          tag="sc" if i % 2 == 0 else "mm",
                                          name="ps_sc")
                        nc.tensor.matmul(ps_sc[:, 0:nn],
                                         t1s[r][:, ch * 128:(ch + 1) * 128],
                                         embed_v[:, n0:n0 + nn],
                                         start=True, stop=True)
                        if i % 2 == 0:
                            nc.vector.tensor_copy(out_sb[:, n0:n0 + nn],
                                                  ps_sc[:, 0:nn])
                        else:
                            nc.scalar.copy(out_sb[:, n0:n0 + nn],
                                           ps_sc[:, 0:nn])
                    seng = nc.scalar if (2 * r + ch) % 2 == 0 else nc.sync
                    seng.dma_start(score_d[r, ch * 128:(ch + 1) * 128, :],
                                   out_sb[:])

    nc.compile()
    return nc


def _prep(inputs):
    """Host-side shard/pad/cast/transpose. Returns in_maps for 8 cores."""
    def padto(a, n, axis):
        pad = [(0, 0)] * a.ndim
        pad[axis] = (0, n - a.shape[axis])
        return np.pad(a, pad)

    import ml_dtypes
    f16 = np.float16
    f32 = np.float32
    f8 = mybir.dt.np(F8)
    fu = padto(padto(np.asarray(inputs['feature_u'], f32), UP, 0), UP, 1)
    fv = padto(padto(np.asarray(inputs['feature_v'], f32), UP, 0), UP, 1)
    Mu = padto(padto(np.asarray(inputs['M_u'], f32), UP, 1), UP, 2)
    Mv = padto(padto(np.asarray(inputs['M_v'], f32), UP, 1), UP, 2)
    W = padto(np.asarray(inputs['W'], f32), UP, 1)
    sfu = padto(np.asarray(inputs['side_feature_u'], f32), UP, 0)
    sfv = padto(np.asarray(inputs['side_feature_v'], f32), UP, 0)
    wcat = np.stack([padto(np.asarray(inputs['w_cat_u'], f32), CAT_BLKS * 128, 0),
                     padto(np.asarray(inputs['w_cat_v'], f32), CAT_BLKS * 128, 0)])
    wcat_r = np.ascontiguousarray(
        wcat.reshape(2, CAT_BLKS, 128, O).transpose(2, 0, 1, 3)).astype(f16)
    wside = np.ascontiguousarray(
        np.stack([np.asarray(inputs['w_side_u'], f32),
                  np.asarray(inputs['w_side_v'], f32)]).transpose(1, 0, 2)
    ).astype(f16)
    # host-computed side-branch BN scale/shift (pure function of inputs;
    # the linear bias cancels inside BN so it is omitted on device too)
    def side_stats(sf, w, g, beta):
        s = np.asarray(sf, np.float64) @ np.asarray(w, np.float64)
        mu = s.mean(0)
        var = s.var(0)
        sc = np.asarray(g, np.float64) / np.sqrt(var + EPS)
        sh = np.asarray(beta, np.float64) - mu * sc
        return sc, sh
    scu, shu = side_stats(inputs['side_feature_u'], inputs['w_side_u'],
                          inputs['g_side_u'], inputs['beta_side_u'])
    scv, shv = side_stats(inputs['side_feature_v'], inputs['w_side_v'],
                          inputs['g_side_v'], inputs['beta_side_v'])
    gbs = np.stack([scu, shu, scv, shv], 1).astype(f32)
    gbc = np.stack([inputs['g_cat_u'], inputs['g_cat_v'],
                    inputs['beta_cat_u'], inputs['beta_cat_v']], 1).astype(f32)
    w2 = np.ascontiguousarray(
        W.reshape(R, KT, 128, H).transpose(2, 1, 0, 3).reshape(128, KT, R * H)
    ).astype(f16)
    q16 = np.ascontiguousarray(
        np.asarray(inputs['Q'], f32).transpose(1, 0, 2)).astype(f16)

    def ktile(a2d):  # [2048, S] -> [128, KT, S] partition-major
        return np.ascontiguousarray(
            a2d.reshape(KT, 128, -1).transpose(1, 0, 2))

    in_maps = []
    for c in range(NC):
        sl = slice(c * S, (c + 1) * S)
        fvT = ktile(fv[sl].T)
        fuT = ktile(fu[sl].T)
        fT = np.concatenate([fvT, fuT], axis=2).astype(f16)
        muT = (np.ascontiguousarray(
            Mu[:, sl, :].transpose(0, 2, 1).reshape(R, KT, 128, S)
            .transpose(0, 2, 1, 3)) * M_SCALE).astype(f8)
        mvT = (np.ascontiguousarray(
            Mv[:, sl, :].transpose(0, 2, 1).reshape(R, KT, 128, S)
            .transpose(0, 2, 1, 3)) * M_SCALE).astype(f8)
        in_maps.append({
            "fT": fT,
            "w2": w2,
            "muT": muT,
            "mvT": mvT,
            "q": q16,
            "sfuT": np.ascontiguousarray(sfu[sl].T).astype(f16),
            "sfvT": np.ascontiguousarray(sfv[sl].T).astype(f16),
            "wside": wside,
            "wcat": wcat_r,
            "gb_side": gbs,
            "gb_cat": gbc,
            "ident": np.eye(128, dtype=f16),
            "mask": np.broadcast_to(
                (np.arange(c * S, (c + 1) * S) < U).astype(f16),
                (SH, S)).copy(),
        })
    return in_maps


def kernel(**inputs) -> np.ndarray:
    if "nc" not in _CACHE:
        _CACHE["nc"] = _build()
    nc = _CACHE["nc"]
    in_maps = _prep(inputs)
    res = bass_utils.run_bass_kernel_spmd(nc, in_maps, core_ids=list(range(NC)))
    score = np.concatenate([res.results[c]["score"] for c in range(NC)], axis=1)
    return score[:, :U, :].astype(np.float32)


if __name__ == "__main__":
    print("kernel module OK")


# revision 76
# speedup vs baseline: 1.0147x; 1.0147x over previous
"""Trainium2 Bass kernel for nn_GCMC (GNN message passing / GCMC scoring).

Strategy: row-shard users AND items across 8 NeuronCores (256 padded rows
each), replicate the small weights. On-chip compute is fp16 (8x less
quantization error than bf16; full PE rate for the cat-layer matmul),
except the message-passing hot loop which runs fp8e4m3 DoubleRow
(pre-activations scaled x16, M matrices x1024; the psum scale is removed
by the relu activation). Side-branch BatchNorm scale/shift are a pure
function of the inputs and are precomputed on the host.

Exactly TWO collectives on the CC stream:
  AG1: both sides' projected features preT, fp8           (164KB in)
  AG3: pre-BN y_v + both sides' cat-BN partial sums as
       fp16 hi/lo pairs (f32-precision sums; fp16-rounded sums make
       E[x^2]-mu^2 cancel catastrophically), fp16          (39.6KB in)
The u-side cat stats ride AG3 so embed_u/t1 compute during the y_v
read-back; embed_v activates per 512-col chunk matching score v-tiles.

All host-side prep (pad/cast/transpose) repacks tensors partition-major
so every device DMA moves multi-KB contiguous lines per partition.
Projections for both sides share one 512-wide moving tile and pair two
relations per 128-wide stationary, quartering instruction count. Gather
read-backs and final-phase loads ride the otherwise idle GPSIMD DMA
queue so the SP/ACT FIFOs and the scalar engine never block the
critical path. Score outputs store as fp16 and are cast to f32 on host.
"""
import sys
if '/opt/trn_rl_repo' not in sys.path:
    sys.path.insert(0, '/opt/trn_rl_repo')

import numpy as np

import concourse.bass as bass
import concourse.bacc as bacc
import concourse.mybir as mybir
import concourse.tile as tile
from concourse import bass_utils

F16 = mybir.dt.float16
F32 = mybir.dt.float32
F8 = mybir.dt.float8e4
AF = mybir.ActivationFunctionType
ALU = mybir.AluOpType
AXX = mybir.AxisListType.X
DR = mybir.MatmulPerfMode.DoubleRow
PRE_SCALE = 16.0     # fp8 scale for staged pre activations
M_SCALE = 1024.0     # fp8 scale for the M matrices

U = V = F = 2000
R, H, O, SH, SF = 5, 64, 75, 64, 128
UP = 2048            # padded U/V/F
S = 256              # rows per core
NC = 8
KT = 16              # 128-row k-tiles over the padded 2048 contraction dims
EPS = 1e-5
CAT_BLKS = 6         # 768 = 6*128 rows of (padded) cat dim; valid rows: 704
NTILES = [(0, 512), (512, 512), (1024, 512), (1536, 464)]  # score v-tiles
SCOLS = R * H        # 320 stage cols: preT only (side BN stats host-computed)
Y3 = S + 8           # 264: y_v (256) + v & u cat-BN sums as fp16 hi/lo pairs

_CACHE = {}


def _build():
    nc = bacc.Bacc("TRN2", target_bir_lowering=False, debug=False,
                   num_devices=NC)

    def din(name, shape, dt):
        return nc.dram_tensor(name, list(shape), dt, kind="ExternalInput").ap()

    fT_d = din("fT", (128, KT, 2 * S), F16)      # [p, k, v256|u256]
    w2_d = din("w2", (128, KT, R * H), F16)      # [p, k, r*64+h]
    muT_d = din("muT", (R, 128, KT, S), F8)      # pre-scaled by M_SCALE
    mvT_d = din("mvT", (R, 128, KT, S), F8)
    q_d = din("q", (O, R, O), F16)
    sfuT_d = din("sfuT", (SF, S), F16)
    sfvT_d = din("sfvT", (SF, S), F16)
    wside_d = din("wside", (SF, 2, SH), F16)
    wcat_d = din("wcat", (128, 2, CAT_BLKS, O), F16)
    gbs_d = din("gb_side", (SH, 4), F32)
    gbc_d = din("gb_cat", (O, 4), F32)
    ident_d = din("ident", (128, 128), F16)
    mask_d = din("mask", (SH, S), F16)

    score_d = nc.dram_tensor("score", [R, S, V], F16, kind="ExternalOutput").ap()

    with tile.TileContext(nc) as tc:
        with tc.tile_pool(name="const", bufs=1) as const_p, \
             tc.tile_pool(name="big", bufs=1) as big_p, \
             tc.tile_pool(name="mstream", bufs=5) as m_p, \
             tc.tile_pool(name="agload", bufs=1) as ag_p, \
             tc.tile_pool(name="small", bufs=1) as sm_p, \
             tc.tile_pool(name="scoresb", bufs=5) as sc_p, \
             tc.tile_pool(name="psmm", bufs=4, space="PSUM") as psmm, \
             tc.tile_pool(name="pssc", bufs=4, space="PSUM") as pssc, \
             tc.tile_pool(name="dram", bufs=1, space="DRAM") as dram_p:

            replica = [list(range(NC))]

            # ============ input loads (SP + ACT queues) ============
            sfvT_sb = const_p.tile([SF, S], F16)
            nc.sync.dma_start(sfvT_sb[:], sfvT_d)
            sfuT_sb = const_p.tile([SF, S], F16)
            nc.sync.dma_start(sfuT_sb[:], sfuT_d)
            wside_sb = const_p.tile([SF, 2, SH], F16)
            nc.sync.dma_start(wside_sb[:], wside_d)
            # w2/fT split across both queues so they get full DMA bandwidth
            # before the bulk M loads start on ACT.
            w2_sb = big_p.tile([128, KT, R * H], F16)
            nc.sync.dma_start(w2_sb[:, 0:KT // 2], w2_d[:, 0:KT // 2])
            nc.scalar.dma_start(w2_sb[:, KT // 2:], w2_d[:, KT // 2:])
            fT_sb = big_p.tile([128, KT, 2 * S], F16)
            nc.sync.dma_start(fT_sb[:, 0:KT // 2], fT_d[:, 0:KT // 2])
            nc.scalar.dma_start(fT_sb[:, KT // 2:], fT_d[:, KT // 2:])
            ident = const_p.tile([128, 128], F16)
            nc.sync.dma_start(ident[:], ident_d)
            mask_sb = const_p.tile([SH, S], F16)
            nc.sync.dma_start(mask_sb[:], mask_d)
            gbs_sb = const_p.tile([SH, 4], F32)
            nc.sync.dma_start(gbs_sb[:], gbs_d)
            gbc_sb = const_p.tile([O, 4], F32)
            nc.sync.dma_start(gbc_sb[:], gbc_d)
            wcat_sb = const_p.tile([128, 2, CAT_BLKS, O], F16)
            nc.sync.dma_start(wcat_sb[:], wcat_d)
            q_sb = const_p.tile([O, R, O], F16)
            nc.sync.dma_start(q_sb[:], q_d)
            eps_t = const_p.tile([128, 1], F32)
            nc.vector.memset(eps_t[:], EPS)

            # ============ bulk M loads (ACT queue, start immediately) ====
            muT_sb = [m_p.tile([128, KT, S], F8, tag="muT", name=f"muT_{r}")
                      for r in range(R)]
            mvT_sb = [m_p.tile([128, KT, S], F8, tag="mvT", name=f"mvT_{r}")
                      for r in range(R)]
            for r in range(R):
                nc.scalar.dma_start(muT_sb[r][:], muT_d[r])
            for r in range(R):
                nc.scalar.dma_start(mvT_sb[r][:], mvT_d[r])

            # ============ collective buffers ============
            # both sides' pre go out in ONE AllGather (payloads are ready
            # together; merging drops one ~8us fixed collective cost)
            ag_in = dram_p.tile([2, 2, 128, SCOLS], F8, name="ag_in")
            ag_out = dram_p.tile([NC, 2, 2, 128, SCOLS], F8,
                                 addr_space="Shared", name="ag_out")
            ag3_in = dram_p.tile([O, Y3], F16, name="ag3_in")
            ag3_out = dram_p.tile([NC, O, Y3], F16, addr_space="Shared",
                                  name="ag3_out")

            # catT: 6 blocks of [128, S] fp16 per side (u=0, v=1)
            catT = [[big_p.tile([128, S], F16, name=f"catT_{sd}_{b}")
                     for b in range(CAT_BLKS)] for sd in range(2)]
            stage = [big_p.tile([128, 2, SCOLS], F8, name=f"stage_{sd}")
                     for sd in range(2)]
            rh_scale = const_p.tile([H, 1], F32)
            nc.vector.memset(rh_scale[:], 1.0 / (PRE_SCALE * M_SCALE))

            def cat_slot(base, r):
                row = base + r * H
                return row // 128, row % 128

            # ============ side branches ============
            # BN stats for the side branch depend only on inputs+weights, so
            # the host precomputes scale/shift (gb_side) - no gather needed.
            s_loc = sm_p.tile([SH, 2, S], F32)

            def side_branch(sd, sfT):
                ps_s = psmm.tile([SH, S], F32, tag="mm", name="ps_side")
                nc.tensor.matmul(ps_s[:], wside_sb[:, sd, :], sfT[:],
                                 start=True, stop=True)
                nc.vector.tensor_copy(s_loc[:, sd, :], ps_s[:])
                nc.scalar.activation(catT[sd][5][0:SH, :], s_loc[:, sd, :],
                                     AF.Relu,
                                     bias=gbs_sb[:, 2 * sd + 1:2 * sd + 2],
                                     scale=gbs_sb[:, 2 * sd:2 * sd + 1])
                nc.vector.tensor_mul(catT[sd][5][0:SH, :],
                                     catT[sd][5][0:SH, :], mask_sb[:])

            side_branch(1, sfvT_sb)
            side_branch(0, sfuT_sb)

            # ============ projections: both sides, paired relations ======
            # psum[rp] [128|64, 512] = [W[2rp]|W[2rp+1]]^T @ [fvT|fuT]
            RPAIRS = [(0, 2), (2, 2), (4, 1)]  # (first r, count)
            ps_rp = []
            for rp, (r0, cnt) in enumerate(RPAIRS):
                ps = psmm.tile([cnt * H, 2 * S], F32, tag="mm",
                               name=f"ps_proj{rp}")
                for k in range(KT):
                    nc.tensor.matmul(ps[:],
                                     w2_sb[:, k, r0 * H:(r0 + cnt) * H],
                                     fT_sb[:, k, :],
                                     start=(k == 0), stop=(k == KT - 1))
                ps_rp.append(ps)
            # copy psum -> catT proj rows for both sides (frees psums)
            for sd in range(2):  # v cols live in 0:S, u cols in S:2S
                col = S if sd == 0 else 0
                for rp, (r0, cnt) in enumerate(RPAIRS):
                    for j in range(cnt):
                        blk, off = cat_slot(320, r0 + j)
                        nc.vector.tensor_copy(
                            catT[sd][blk][off:off + H, :],
                            ps_rp[rp][j * H:(j + 1) * H, col:col + S])

            # transpose preT -> natural [v, h] chunks, stage, gather
            def stage_side(sd):
                for r in range(R):
                    blk, off = cat_slot(320, r)
                    for ch in range(2):
                        ps_tp = psmm.tile([128, H], F16, tag="mm", name="ps_tp")
                        nc.tensor.transpose(
                            ps_tp[:],
                            catT[sd][blk][off:off + H, ch * 128:(ch + 1) * 128],
                            ident[off:off + H, off:off + H])
                        nc.vector.tensor_scalar_mul(
                            stage[sd][:, ch, r * H:(r + 1) * H], ps_tp[:],
                            PRE_SCALE)
                nc.sync.dma_start(ag_in[sd].rearrange("c p j -> p c j"),
                                  stage[sd][:])

            stage_side(1)
            stage_side(0)
            nc.gpsimd.collective_compute("AllGather", ALU.bypass,
                                         replica_groups=replica,
                                         ins=[ag_in.opt()],
                                         outs=[ag_out.opt()])

            # ============ gathered pre-activations ============
            # agall[sd] [128, NC, 2, SCOLS]; k-chunk kk -> [:, kk//2, kk%2, :]
            agall = [ag_p.tile([128, NC, 2, SCOLS], F8, name=f"agall{sd}")
                     for sd in range(2)]

            def load_agall(sd):
                # v-gather reads on SP queue; u-gather reads on the otherwise
                # idle GPSIMD queue so neither the SP FIFO (ag3 stages) nor
                # the scalar engine (hidden relu activations) is blocked.
                eng = nc.sync if sd == 1 else nc.gpsimd
                for c in range(NC):
                    eng.dma_start(
                        agall[sd][:, c],
                        ag_out[c, sd].rearrange("ch p j -> p ch j"))

            # ============ BN helpers ============
            def bn_from_sums(sums, sumsq, g_col, b_col, n, P, W=1):
                mu = sm_p.tile([P, W], F32, tag="bn_mu", name="bn_mu")
                nc.vector.tensor_scalar_mul(mu[:], sums[:], 1.0 / n)
                e2 = sm_p.tile([P, W], F32, tag="bn_e2", name="bn_e2")
                nc.vector.tensor_scalar_mul(e2[:], sumsq[:], 1.0 / n)
                var = sm_p.tile([P, W], F32, tag="bn_var", name="bn_var")
                nc.vector.tensor_mul(var[:], mu[:], mu[:])
                nc.vector.tensor_sub(var[:], e2[:], var[:])
                std = sm_p.tile([P, W], F32, tag="bn_std", name="bn_std")
                nc.scalar.activation(std[:], var[:], AF.Sqrt, bias=eps_t[0:P, :])
                rstd = sm_p.tile([P, W], F32, tag="bn_rstd", name="bn_rstd")
                nc.vector.reciprocal(rstd[:], std[:])
                scale = sm_p.tile([P, W], F32, tag="bn_scale", name="bn_scale")
                nc.vector.tensor_mul(scale[:], g_col, rstd[:])
                shift = sm_p.tile([P, W], F32, tag="bn_shift", name="bn_shift")
                nc.vector.tensor_mul(shift[:], mu[:], scale[:])
                nc.vector.tensor_sub(shift[:], b_col, shift[:])
                return scale, shift

            # ============ hidden: relu(pre_all^T @ MT) -> catT rows 0:320 ====
            # fp8 DoubleRow: each matmul consumes a (c, ch) k-tile PAIR at
            # double rate; psum carries PRE_SCALE*M_SCALE, removed by the
            # relu activation's scale.
            def hidden_side(sd, osd, mT):
                for r in range(R):
                    ps_h = psmm.tile([H, S], F32, tag="mm", name="ps_h")
                    for c in range(NC):
                        nc.tensor.matmul(
                            ps_h[:],
                            agall[osd][:, c, :, r * H:(r + 1) * H],
                            mT[r][:, 2 * c:2 * c + 2, :],
                            start=(c == 0), stop=(c == NC - 1),
                            perf_mode=DR)
                    blk, off = cat_slot(0, r)
                    nc.scalar.activation(catT[sd][blk][off:off + H, :],
                                         ps_h[:], AF.Relu, scale=rh_scale[:])

            # ============ cat matmul (fp16) + y stats ============
            ysb = sm_p.tile([O, 2, S], F32)
            junk_y = sm_p.tile([O, 2, S], F32, name="junk_y")
            ag3_sb = sm_p.tile([O, Y3], F16)

            def cat_side(sd):
                # f32 sums ride the fp16 gather as hi/lo pairs: the BN
                # variance E[x^2]-mu^2 cancels catastrophically with
                # fp16-rounded sums (relu'd features: mean >> std).
                # cols: 256:258 v-sums hi, 258:260 v lo, 260:262 u hi,
                # 262:264 u lo.
                dst = ag3_sb
                base = S + (4 if sd == 0 else 0)
                ps_y = psmm.tile([O, S], F32, tag="mm", name="ps_y")
                for b in range(CAT_BLKS):
                    kk = 128 if b < 5 else 64
                    nc.tensor.matmul(ps_y[:], wcat_sb[0:kk, sd, b, :],
                                     catT[sd][b][0:kk, :],
                                     start=(b == 0), stop=(b == CAT_BLKS - 1))
                if sd == 0:
                    nc.vector.tensor_copy(ysb[:, 0, :], ps_y[:])
                else:
                    # y_v goes straight psum -> fp16 payload on the scalar
                    # engine; sums read the psum directly on vector.
                    nc.scalar.copy(ag3_sb[:, 0:S], ps_y[:])
                nc.scalar.activation(junk_y[:, sd, :], ps_y[:], AF.Square)
                yss = sm_p.tile([O, 2], F32, tag=f"yss_{sd}", name="yss")
                nc.vector.reduce_sum(yss[:, 0:1], ps_y[:], axis=AXX)
                nc.vector.reduce_sum(yss[:, 1:2], junk_y[:, sd, :], axis=AXX)
                hi_f = sm_p.tile([O, 2], F32, tag=f"hi_f{sd}", name="hi_f")
                lo_f = sm_p.tile([O, 2], F32, tag=f"lo_f{sd}", name="lo_f")
                nc.vector.tensor_copy(dst[:, base:base + 2], yss[:])
                nc.vector.tensor_copy(hi_f[:], dst[:, base:base + 2])
                nc.vector.tensor_sub(lo_f[:], yss[:], hi_f[:])
                nc.vector.tensor_copy(dst[:, base + 2:base + 4], lo_f[:])

            def half_stats(yfx, c0, c1, g_col, b_col):
                s1 = sm_p.tile([O, 1], F32, tag=f"s1_{c0}", name="s1")
                q1 = sm_p.tile([O, 1], F32, tag=f"q1_{c0}", name="q1")
                sl = sm_p.tile([O, 1], F32, tag=f"sl_{c0}", name="sl")
                ql = sm_p.tile([O, 1], F32, tag=f"ql_{c0}", name="ql")
                nc.vector.reduce_sum(s1[:], yfx[:, :, c0], axis=AXX)
                nc.vector.reduce_sum(q1[:], yfx[:, :, c0 + 1], axis=AXX)
                nc.vector.reduce_sum(sl[:], yfx[:, :, c1], axis=AXX)
                nc.vector.reduce_sum(ql[:], yfx[:, :, c1 + 1], axis=AXX)
                nc.vector.tensor_add(s1[:], s1[:], sl[:])
                nc.vector.tensor_add(q1[:], q1[:], ql[:])
                return bn_from_sums(s1, q1, g_col, b_col, U, O)

            # ============ main sequence ============
            load_agall(1)
            load_agall(0)
            hidden_side(0, 1, muT_sb)
            cat_side(0)
            hidden_side(1, 0, mvT_sb)
            cat_side(1)
            nc.sync.dma_start(ag3_in[:], ag3_sb[:])
            nc.gpsimd.collective_compute("AllGather", ALU.bypass,
                                         replica_groups=replica,
                                         ins=[ag3_in.opt()],
                                         outs=[ag3_out.opt()])
            # ============ stats -> embeds -> t1 ============
            # sums columns land first; u chain runs first so embed_u/t1
            # overlap the y_v chunk read-back.
            yf = sm_p.tile([O, NC, Y3], F16, name="yf")
            nc.gpsimd.dma_start(yf[:, :, S:],
                                ag3_out[:, :, S:].rearrange("c p j -> p c j"))
            sc_u, sh_u = half_stats(yf, S + 4, S + 6, gbc_sb[:, 0:1],
                                    gbc_sb[:, 2:3])
            embed_u = sm_p.tile([O, S], F16)
            nc.scalar.activation(embed_u[:], ysb[:, 0, :],
                                 AF.Relu, bias=sh_u[:], scale=sc_u[:])
            t1s = []
            for r in range(R):
                ps_t1 = psmm.tile([O, S], F32, tag="mm", name="ps_t1")
                nc.tensor.matmul(ps_t1[:], q_sb[:, r, :], embed_u[:],
                                 start=True, stop=True)
                t1 = sm_p.tile([O, S], F16, tag=f"t1_{r}", name=f"t1_{r}")
                nc.vector.tensor_copy(t1[:], ps_t1[:])
                t1s.append(t1)

            embed_v = sm_p.tile([O, UP], F16)
            sc_v, sh_v = half_stats(yf, S, S + 2, gbc_sb[:, 1:2],
                                    gbc_sb[:, 3:4])
            embed_v4 = embed_v.rearrange("p (c u) -> p c u", c=NC)
            for j in range(4):
                nc.gpsimd.dma_start(
                    yf[:, 2 * j:2 * j + 2, 0:S],
                    ag3_out[2 * j:2 * j + 2, :, 0:S]
                    .rearrange("c p j -> p c j"))
                nc.scalar.activation(
                    embed_v4[:, 2 * j:2 * j + 2, :],
                    yf[:, 2 * j:2 * j + 2, 0:S],
                    AF.Relu, bias=sh_v[:], scale=sc_v[:])

            # ============ score ============
            for r in range(R):
                for ch in range(2):
                    out_sb = sc_p.tile([128, V], F16, tag="osb", name="out_sb")
                    for i, (n0, nn) in enumerate(NTILES):
                        pool = pssc if i % 2 == 0 else psmm
                        ps_sc = pool.tile([128, 512], F32,
                                          tag="sc" if i % 2 == 0 else "mm",
                                          name="ps_sc")
                        nc.tensor.matmul(ps_sc[:, 0:nn],
                                         t1s[r][:, ch * 128:(ch + 1) * 128],
                                         embed_v[:, n0:n0 + nn],
                                         start=True, stop=True)
                        if i % 2 == 0:
                            nc.vector.tensor_copy(out_sb[:, n0:n0 + nn],
                                                  ps_sc[:, 0:nn])
                        else:
                            nc.scalar.copy(out_sb[:, n0:n0 + nn],
                                           ps_sc[:, 0:nn])
                    seng = nc.scalar if (2 * r + ch) % 2 == 0 else nc.sync
                    seng.dma_start(score_d[r, ch * 128:(ch + 1) * 128, :],
                                   out_sb[:])

    nc.compile()
    return nc


def _prep(inputs):
    """Host-side shard/pad/cast/transpose. Returns in_maps for 8 cores."""
    def padto(a, n, axis):
        pad = [(0, 0)] * a.ndim
        pad[axis] = (0, n - a.shape[axis])
        return np.pad(a, pad)

    import ml_dtypes
    f16 = np.float16
    f32 = np.float32
    f8 = mybir.dt.np(F8)
    fu = padto(padto(np.asarray(inputs['feature_u'], f32), UP, 0), UP, 1)
    fv = padto(padto(np.asarray(inputs['feature_v'], f32), UP, 0), UP, 1)
    Mu = padto(padto(np.asarray(inputs['M_u'], f32), UP, 1), UP, 2)
    Mv = padto(padto(np.asarray(inputs['M_v'], f32), UP, 1), UP, 2)
    W = padto(np.asarray(inputs['W'], f32), UP, 1)
    sfu = padto(np.asarray(inputs['side_feature_u'], f32), UP, 0)
    sfv = padto(np.asarray(inputs['side_feature_v'], f32), UP, 0)
    wcat = np.stack([padto(np.asarray(inputs['w_cat_u'], f32), CAT_BLKS * 128, 0),
                     padto(np.asarray(inputs['w_cat_v'], f32), CAT_BLKS * 128, 0)])
    wcat_r = np.ascontiguousarray(
        wcat.reshape(2, CAT_BLKS, 128, O).transpose(2, 0, 1, 3)).astype(f16)
    wside = np.ascontiguousarray(
        np.stack([np.asarray(inputs['w_side_u'], f32),
                  np.asarray(inputs['w_side_v'], f32)]).transpose(1, 0, 2)
    ).astype(f16)
    # host-computed side-branch BN scale/shift (pure function of inputs;
    # the linear bias cancels inside BN so it is omitted on device too)
    def side_stats(sf, w, g, beta):
        s = np.asarray(sf, np.float64) @ np.asarray(w, np.float64)
        mu = s.mean(0)
        var = s.var(0)
        sc = np.asarray(g, np.float64) / np.sqrt(var + EPS)
        sh = np.asarray(beta, np.float64) - mu * sc
        return sc, sh
    scu, shu = side_stats(inputs['side_feature_u'], inputs['w_side_u'],
                          inputs['g_side_u'], inputs['beta_side_u'])
    scv, shv = side_stats(inputs['side_feature_v'], inputs['w_side_v'],
                          inputs['g_side_v'], inputs['beta_side_v'])
    gbs = np.stack([scu, shu, scv, shv], 1).astype(f32)
    gbc = np.stack([inputs['g_cat_u'], inputs['g_cat_v'],
                    inputs['beta_cat_u'], inputs['beta_cat_v']], 1).astype(f32)
    w2 = np.ascontiguousarray(
        W.reshape(R, KT, 128, H).transpose(2, 1, 0, 3).reshape(128, KT, R * H)
    ).astype(f16)
    q16 = np.ascontiguousarray(
        np.asarray(inputs['Q'], f32).transpose(1, 0, 2)).astype(f16)

    def ktile(a2d):  # [2048, S] -> [128, KT, S] partition-major
        return np.ascontiguousarray(
            a2d.reshape(KT, 128, -1).transpose(1, 0, 2))

    in_maps = []
    for c in range(NC):
        sl = slice(c * S, (c + 1) * S)
        fvT = ktile(fv[sl].T)
        fuT = ktile(fu[sl].T)
        fT = np.concatenate([fvT, fuT], axis=2).astype(f16)
        muT = (np.ascontiguousarray(
            Mu[:, sl, :].transpose(0, 2, 1).reshape(R, KT, 128, S)
            .transpose(0, 2, 1, 3)) * M_SCALE).astype(f8)
        mvT = (np.ascontiguousarray(
            Mv[:, sl, :].transpose(0, 2, 1).reshape(R, KT, 128, S)
            .transpose(0, 2, 1, 3)) * M_SCALE).astype(f8)
        in_maps.append({
            "fT": fT,
            "w2": w2,
            "muT": muT,
            "mvT": mvT,
            "q": q16,
            "sfuT": np.ascontiguousarray(sfu[sl].T).astype(f16),
            "sfvT": np.ascontiguousarray(sfv[sl].T).astype(f16),
            "wside": wside,
            "wcat": wcat_r,
            "gb_side": gbs,
            "gb_cat": gbc,
            "ident": np.eye(128, dtype=f16),
            "mask": np.broadcast_to(
                (np.arange(c * S, (c + 1) * S) < U).astype(f16),
                (SH, S)).copy(),
        })
    return in_maps


def kernel(**inputs) -> np.ndarray:
    if "nc" not in _CACHE:
        _CACHE["nc"] = _build()
    nc = _CACHE["nc"]
    in_maps = _prep(inputs)
    res = bass_utils.run_bass_kernel_spmd(nc, in_maps, core_ids=list(range(NC)))
    score = np.concatenate([res.results[c]["score"] for c in range(NC)], axis=1)
    return score[:, :U, :].astype(np.float32)


if __name__ == "__main__":
    print("kernel module OK")
